# revision 1
# baseline (speedup 1.0000x reference)
"""TRN2 Bass kernel for nn_CharModel (segment-mean over char ranges + pos embedding).

Strategy (pure data-parallel over batch, 8 cores x 4 batches):
  - Host computes per-word [start, end) ranges exactly as the reference does and
    sorts each batch's words by length descending, so every gather step's
    participant set ("len >= threshold") is a dense slot prefix.
  - Device: per batch a few SWDGE dma_gather steps land char rows in slot order
    [p = i%128, chunk = i//128]:
      * odd step first: row start+len-1 for odd-len words gathered straight
        into the accumulator (even-len / padding words point at zeros rows
        appended to the per-core feats copy, so every slot is initialized)
      * pair step t: rows (start+2t, start+2t+1) for words with len >= 2t+2 --
        one 6KB descriptor per word (elem_size=2*D over an elem_step=D
        overlapping view); DVE prefix-adds fold the stages into the accumulator.
    Total gathered bytes ~= one pass over feats (memory roofline).
  - Pos embedding via a host-built one-hot matmul on PE (PSUM), fused with the
    1/len scaling in one scalar_tensor_tensor per 128-word chunk:
       out = (acc * recip) + psum_pos
  - Host unpermutes word slots and stacks cores.
"""

import numpy as np

B, S, W, D, PV = 32, 2048, 512, 768, 64
N_CORES = 8
BPC = B // N_CORES          # batches per core
P = 128
C = W // P                  # 4 word-chunks per batch
ZROW = BPC * S              # first zeros row in feats_cat
NZROWS = 40                 # spread pad reads across many zero rows
KMAX_DEVICE = 48            # device path supports word len up to this

LAST_RESULTS = None         # BassKernelResults of the most recent run (for test.py)


def _run_spmd(nc, in_maps, core_ids):
    """Indirection point so tests can swap in a simulator."""
    from concourse.bass_utils import run_bass_kernel_spmd
    return run_bass_kernel_spmd(nc, in_maps, core_ids)


def _word_ranges(word_lens, pos, seq_len):
    """Replicate the reference's starts/ends/valid computation in numpy."""
    wl = np.asarray(word_lens, np.int64)
    po = np.asarray(pos, np.int64)
    sl = np.asarray(seq_len, np.int64)
    b, w = wl.shape
    j = np.arange(w)
    next_start = np.concatenate([wl[:, 1:], np.zeros((b, 1), np.int64)], axis=1)
    is_last = (j[None, :] == w - 1) | (next_start == 0)
    starts = wl
    ends = np.where(is_last, sl[:, None], next_start)
    valid = (wl != 0) | (j[None, :] == 0)
    lens = np.where(valid, np.maximum(ends - starts, 0), 0)
    denom = np.maximum(ends - starts, 1).astype(np.float64)
    recip = np.where(valid & (lens > 0), 1.0 / denom, 0.0).astype(np.float32)
    return starts, lens, recip, po


def _numpy_fallback(feats, pos_table, word_lens, pos, seq_len):
    feats = np.asarray(feats, np.float32)
    pos_table = np.asarray(pos_table, np.float32)
    starts, lens, recip, po = _word_ranges(word_lens, pos, seq_len)
    out = np.zeros((feats.shape[0], po.shape[1], feats.shape[2]), np.float32)
    for b in range(out.shape[0]):
        for w in range(out.shape[1]):
            L = int(lens[b, w])
            if L > 0:
                s = int(starts[b, w])
                out[b, w] = feats[b, s:s + L].sum(axis=0) * recip[b, w]
        out[b] += pos_table[po[b]]
    return out


def _wrap16(flat):
    """int16 flat index list [W] -> the q7 kernel's [16, W/16] wrapped layout,
    replicated across the 8 q7 core stripes (128 partitions)."""
    wrapped = flat.astype(np.int16).reshape(-1, 16).T
    return np.tile(wrapped, (8, 1))


def _concourse_importable():
    try:
        import concourse.bass  # noqa: F401
        return True
    except ImportError:
        import sys
        for p in ("/opt/trn_rl_repo", "/root/.axon_site/_ro/trn_rl_repo"):
            if p not in sys.path:
                sys.path.append(p)
        try:
            import concourse.bass  # noqa: F401
            return True
        except ImportError:
            return False


def kernel(feats, pos_table, word_lens, pos, seq_len):
    global LAST_RESULTS
    feats = np.ascontiguousarray(np.asarray(feats, np.float32))
    pos_table_np = np.ascontiguousarray(np.asarray(pos_table, np.float32))
    starts, lens, recip, po = _word_ranges(word_lens, pos, seq_len)

    kmax = int(lens.max())
    shapes_ok = (
        feats.shape == (B, S, D)
        and pos_table_np.shape == (PV, D)
        and po.shape == (B, W)
        and starts.shape == (B, W)
        and np.asarray(seq_len).shape == (B,)
        and int(po.max()) < PV and int(po.min()) >= 0
    )
    if kmax > KMAX_DEVICE or not shapes_ok or not _concourse_importable():
        return _numpy_fallback(feats, pos_table, word_lens, pos, seq_len)
    kmax = max(kmax, 1)
    n_pair = kmax // 2                         # pair step t covers rows 2t,2t+1

    # ---- host-side per-core tensors -------------------------------------
    perms = np.zeros((B, W), np.int64)                 # slot i -> word perms[b, i]
    pair_n = np.zeros((B, max(n_pair, 1)), np.int64)   # words with len >= 2t+2
    any_odd = np.zeros(B, bool)
    for b in range(B):
        perm = np.argsort(-lens[b], kind="stable")
        perms[b] = perm
        sl = lens[b][perm]
        for t in range(n_pair):
            pair_n[b, t] = int((sl >= 2 * t + 2).sum())
        any_odd[b] = bool((lens[b] % 2 == 1).any())
    pair_n_u = pair_n.reshape(N_CORES, BPC, -1).max(axis=0)   # [BPC, n_pair]
    odd_u = any_odd.reshape(N_CORES, BPC).any(axis=0)          # [BPC]

    idx_cols = W // 16                         # 32 int16 columns per step
    n_steps = n_pair + 1                       # odd step slot + pair steps
    in_maps = []
    host_meta = []
    for core in range(N_CORES):
        bs = slice(core * BPC, (core + 1) * BPC)
        feats_cat = np.zeros((BPC * S + NZROWS, D), np.float32)
        feats_cat[:BPC * S] = feats[bs].reshape(-1, D)

        idx_all = np.full((128, BPC * n_steps * idx_cols), -1, np.int16)
        recip_all = np.zeros((P, BPC * C), np.float32)
        onehot_all = np.zeros((PV, BPC * W), np.float32)
        for bl in range(BPC):
            bg = core * BPC + bl
            perm = perms[bg]
            st = starts[bg][perm]
            ln = lens[bg][perm]
            odd = (ln % 2 == 1)
            zcycle = ZROW + (np.arange(W) % NZROWS)
            flat = np.where(odd, bl * S + st + ln - 1, zcycle)
            col0 = (bl * n_steps) * idx_cols
            idx_all[:, col0:col0 + idx_cols] = _wrap16(flat)
            for t in range(n_pair):
                nn = int(pair_n_u[bl, t])
                if nn == 0:
                    continue
                nv = int(pair_n[bg, t])
                flat = np.full(W, -1, np.int64)
                flat[:nv] = bl * S + st[:nv] + 2 * t
                flat[nv:nn] = ZROW + (np.arange(nn - nv) % (NZROWS - 1))
                col0 = (bl * n_steps + 1 + t) * idx_cols
                idx_all[:, col0:col0 + idx_cols] = _wrap16(flat)
            slot_r = recip[bg][perm]              # [512] in slot order
            recip_all[:, bl * C:(bl + 1) * C] = slot_r.reshape(C, P).T
            slot_pos = po[bg][perm]
            onehot_all[slot_pos, bl * W + np.arange(W)] = 1.0

        # batch-0 odd-step indices in [128, C] int32 column layout for the
        # library-free indirect gather that warms up under the q7 library load
        bg0 = core * BPC
        perm0 = perms[bg0]
        ln0 = lens[bg0][perm0]
        st0 = starts[bg0][perm0]
        odd0 = (ln0 % 2 == 1)
        zc0 = ZROW + (np.arange(W) % NZROWS)
        flat0 = np.where(odd0, st0 + ln0 - 1, zc0)
        odd0_idx = flat0.reshape(C, P).T.astype(np.int32)

        in_maps.append({
            "feats_cat": feats_cat,
            "pos_tab": pos_table_np,
            "idx_all": idx_all,
            "recip_all": recip_all,
            "onehot_all": onehot_all,
            "odd0_idx": odd0_idx,
        })
        host_meta.append(perms[bs])

    # ---- device program --------------------------------------------------
    from concourse import bass, bacc, mybir
    import concourse.tile as tile

    nc = bacc.Bacc("TRN2", target_bir_lowering=False, debug=False)
    t_feats = nc.dram_tensor("feats_cat", [BPC * S + NZROWS, D], mybir.dt.float32,
                             kind="ExternalInput")
    t_pos = nc.dram_tensor("pos_tab", [PV, D], mybir.dt.float32,
                           kind="ExternalInput")
    t_idx = nc.dram_tensor("idx_all", [128, BPC * n_steps * idx_cols],
                           mybir.dt.int16, kind="ExternalInput")
    t_recip = nc.dram_tensor("recip_all", [P, BPC * C], mybir.dt.float32,
                             kind="ExternalInput")
    t_oh = nc.dram_tensor("onehot_all", [PV, BPC * W], mybir.dt.float32,
                          kind="ExternalInput")
    t_odd0 = nc.dram_tensor("odd0_idx", [P, C], mybir.dt.int32,
                            kind="ExternalInput")
    t_out = nc.dram_tensor("out", [BPC, P, C * D], mybir.dt.float32,
                           kind="ExternalOutput")

    # overlapping pair view: index i -> 2*D consecutive elements (rows i, i+1)
    feats_ap = t_feats[:]
    pair_src = bass.AP(feats_ap.tensor, 0,
                       [[D, BPC * S + NZROWS - 1], [1, 2 * D]])

    with tile.TileContext(nc) as tc:
        with (
            tc.tile_pool(name="const", bufs=1) as cpool,
            tc.tile_pool(name="work", bufs=3) as wpool,
            tc.tile_pool(name="stage", bufs=5) as spool,
            tc.tile_pool(name="psum", bufs=4, space="PSUM") as ppool,
        ):
            pos_sb = cpool.tile([PV, D], mybir.dt.float32)
            oh_sb = cpool.tile([PV, BPC * W], mybir.dt.float32)
            recip_sb = cpool.tile([P, BPC * C], mybir.dt.float32)
            idx_sb = cpool.tile([128, BPC * n_steps * idx_cols], mybir.dt.int16)
            odd0_sb = cpool.tile([P, C], mybir.dt.int32)
            nc.sync.dma_start(out=odd0_sb[:], in_=t_odd0[:])
            nc.sync.dma_start(out=idx_sb[:], in_=t_idx[:])
            nc.sync.dma_start(out=pos_sb[:], in_=t_pos[:])
            nc.sync.dma_start(out=oh_sb[:], in_=t_oh[:])
            nc.sync.dma_start(out=recip_sb[:], in_=t_recip[:])

            accs = {}

            def issue_odd(bl):
                acc = wpool.tile([P, C, D], mybir.dt.float32, tag="acc")
                accs[bl] = acc
                if bl == 0:
                    # library-free indirect gathers: run while the q7 dma_gather
                    # library is still being fetched
                    for c in range(C):
                        nc.gpsimd.indirect_dma_start(
                            out=acc[:, c, :],
                            out_offset=None,
                            in_=t_feats[:],
                            in_offset=bass.IndirectOffsetOnAxis(
                                ap=odd0_sb[:, c:c + 1], axis=0
                            ),
                        )
                    return
                nc.gpsimd.dma_gather(
                    acc[:],
                    t_feats[:],
                    idx_sb[:, bl * n_steps * idx_cols:
                           bl * n_steps * idx_cols + idx_cols],
                    W, W, D, single_packet=False,
                )

            def issue_pair(bl, t):
                nn = int(pair_n_u[bl, t])
                if nn == 0:
                    return
                acc = accs[bl]
                stg = spool.tile([P, C, 2 * D], mybir.dt.float32, tag="stg2")
                colk = (bl * n_steps + 1 + t) * idx_cols
                nc.gpsimd.dma_gather(
                    stg[:],
                    pair_src,
                    idx_sb[:, colk:colk + idx_cols],
                    W, nn, 2 * D, elem_step=D, single_packet=False,
                )
                fc, rem = nn // P, nn % P
                if fc:
                    nc.vector.tensor_add(
                        out=acc[:, 0:fc, :], in0=acc[:, 0:fc, :],
                        in1=stg[:, 0:fc, 0:D],
                    )
                    nc.vector.tensor_add(
                        out=acc[:, 0:fc, :], in0=acc[:, 0:fc, :],
                        in1=stg[:, 0:fc, D:2 * D],
                    )
                if rem:
                    nc.vector.tensor_add(
                        out=acc[0:rem, fc, :], in0=acc[0:rem, fc, :],
                        in1=stg[0:rem, fc, 0:D],
                    )
                    nc.vector.tensor_add(
                        out=acc[0:rem, fc, :], in0=acc[0:rem, fc, :],
                        in1=stg[0:rem, fc, D:2 * D],
                    )

            def issue_epilogue(bl):
                acc = accs[bl]
                for c in range(C):
                    psum = ppool.tile([P, D], mybir.dt.float32, space="PSUM",
                                      tag="psum")
                    lhs = oh_sb[:, bl * W + c * P: bl * W + (c + 1) * P]
                    nc.tensor.matmul(out=psum[:, 0:512], lhsT=lhs,
                                     rhs=pos_sb[:, 0:512], start=True, stop=True)
                    nc.tensor.matmul(out=psum[:, 512:D], lhsT=lhs,
                                     rhs=pos_sb[:, 512:D], start=True, stop=True)
                    nc.vector.scalar_tensor_tensor(
                        out=acc[:, c, :],
                        in0=acc[:, c, :],
                        scalar=recip_sb[:, bl * C + c: bl * C + c + 1],
                        in1=psum[:],
                        op0=mybir.AluOpType.mult,
                        op1=mybir.AluOpType.add,
                    )
                    # store each chunk as soon as its epilogue op retires, so
                    # only a 384KB DMA (not 1.5MB) trails the last compute
                    nc.sync.dma_start(
                        out=t_out[bl, :, c * D:(c + 1) * D],
                        in_=acc[:, c, :],
                    )

            # Batches 0..BPC-3 run batch-major; the last two batches interleave
            # their gathers so the final batch's add-chain starts while the
            # remaining gathers drain, shrinking the end-of-kernel tail.
            for bl in range(BPC - 2):
                issue_odd(bl)
                for t in range(n_pair):
                    issue_pair(bl, t)
                issue_epilogue(bl)
            if BPC >= 2:
                a, b = BPC - 2, BPC - 1
                issue_odd(a)
                issue_pair(a, 0)
                issue_odd(b)
                issue_pair(b, 0)
                for t in range(1, n_pair):
                    issue_pair(a, t)
                issue_epilogue(a)
                for t in range(1, n_pair):
                    issue_pair(b, t)
                issue_epilogue(b)
    nc.finalize()

    res = _run_spmd(nc, in_maps, list(range(N_CORES)))
    LAST_RESULTS = res

    out = np.empty((B, W, D), np.float32)
    for core in range(N_CORES):
        arr = res.results[core]["out"]            # [BPC, 128, C*D]
        for bl in range(BPC):
            slots = arr[bl].reshape(P, C, D).transpose(1, 0, 2).reshape(W, D)
            perm = host_meta[core][bl]
            out[core * BPC + bl][perm] = slots
    return out



# revision 2
# speedup vs baseline: 1.4129x; 1.4129x over previous
"""TRN2 Bass kernel for nn_CharModel (segment-mean over char ranges + pos embedding).

Strategy (pure data-parallel over batch, 8 cores x 4 batches, all bf16):
  - Host computes per-word [start, end) ranges exactly as the reference does,
    sorts each batch's words by length descending (so each 128-word chunk has a
    small max length = its gather window), and converts feats to bf16.
  - Device, per (batch, chunk): ONE hardware-DGE indirect DMA gathers a
    contiguous `wlen`-row window (wlen = chunk max word len) starting at each
    word's first char row -- one ~wlen*1.5KB contiguous descriptor per word.
    A chain of `wlen` scalar_tensor_tensor ops folds the window into the mean:
        acc = g[:,k,:] * m[:,k] + acc      (m[p,k] = recip_p if k < len_p else 0)
    so garbage rows are multiplied by zero -- no zero-fill traffic, exact
    per-word masking. The chain is seeded with the pos-embedding PSUM tile
    (bf16 one-hot matmul), so the epilogue is free.
  - Output is written bf16 and upcast on host; host unpermutes word slots.
"""

import numpy as np
import ml_dtypes

B, S, W, D, PV = 32, 2048, 512, 768, 64
N_CORES = 8
BPC = B // N_CORES          # batches per core
P = 128
C = W // P                  # 4 word-chunks per batch
PAD_ROWS = 8                # window over-read room past the last batch
KMAX_DEVICE = 16            # device path supports chunk windows up to this

LAST_RESULTS = None         # BassKernelResults of the most recent run (for test.py)

BF16 = ml_dtypes.bfloat16


def _run_spmd(nc, in_maps, core_ids):
    """Indirection point so tests can swap in a simulator."""
    from concourse.bass_utils import run_bass_kernel_spmd
    return run_bass_kernel_spmd(nc, in_maps, core_ids)


def _word_ranges(word_lens, pos, seq_len):
    """Replicate the reference's starts/ends/valid computation in numpy."""
    wl = np.asarray(word_lens, np.int64)
    po = np.asarray(pos, np.int64)
    sl = np.asarray(seq_len, np.int64)
    b, w = wl.shape
    j = np.arange(w)
    next_start = np.concatenate([wl[:, 1:], np.zeros((b, 1), np.int64)], axis=1)
    is_last = (j[None, :] == w - 1) | (next_start == 0)
    starts = wl
    ends = np.where(is_last, sl[:, None], next_start)
    valid = (wl != 0) | (j[None, :] == 0)
    lens = np.where(valid, np.maximum(ends - starts, 0), 0)
    denom = np.maximum(ends - starts, 1).astype(np.float64)
    recip = np.where(valid & (lens > 0), 1.0 / denom, 0.0).astype(np.float32)
    return starts, lens, recip, po


def _numpy_fallback(feats, pos_table, word_lens, pos, seq_len):
    feats = np.asarray(feats, np.float32)
    pos_table = np.asarray(pos_table, np.float32)
    starts, lens, recip, po = _word_ranges(word_lens, pos, seq_len)
    out = np.zeros((feats.shape[0], po.shape[1], feats.shape[2]), np.float32)
    for b in range(out.shape[0]):
        for w in range(out.shape[1]):
            L = int(lens[b, w])
            if L > 0:
                s = int(starts[b, w])
                out[b, w] = feats[b, s:s + L].sum(axis=0) * recip[b, w]
        out[b] += pos_table[po[b]]
    return out


def _concourse_importable():
    try:
        import concourse.bass  # noqa: F401
        return True
    except ImportError:
        import sys
        for p in ("/opt/trn_rl_repo", "/root/.axon_site/_ro/trn_rl_repo"):
            if p not in sys.path:
                sys.path.append(p)
        try:
            import concourse.bass  # noqa: F401
            return True
        except ImportError:
            return False


def kernel(feats, pos_table, word_lens, pos, seq_len):
    global LAST_RESULTS
    feats = np.ascontiguousarray(np.asarray(feats, np.float32))
    pos_table_np = np.ascontiguousarray(np.asarray(pos_table, np.float32))
    starts, lens, recip, po = _word_ranges(word_lens, pos, seq_len)

    shapes_ok = (
        feats.shape == (B, S, D)
        and pos_table_np.shape == (PV, D)
        and po.shape == (B, W)
        and starts.shape == (B, W)
        and np.asarray(seq_len).shape == (B,)
        and int(po.max()) < PV and int(po.min()) >= 0
    )
    if not shapes_ok or not _concourse_importable():
        return _numpy_fallback(feats, pos_table, word_lens, pos, seq_len)

    # ---- host-side slot assignment ---------------------------------------
    perms = np.zeros((B, W), np.int64)              # slot i -> word perms[b, i]
    sl_sorted = np.zeros((B, W), np.int64)
    for b in range(B):
        perm = np.argsort(-lens[b], kind="stable")
        perms[b] = perm
        sl_sorted[b] = lens[b][perm]
    # per (batch, chunk) max word length; unified across cores (shared program)
    chunk_max = sl_sorted.reshape(B, C, P).max(axis=2)            # [B, C]
    wlen_u = chunk_max.reshape(N_CORES, BPC, C).max(axis=0)       # [BPC, C]
    wlen_u = np.maximum(wlen_u, 1)
    if int(wlen_u.max()) > KMAX_DEVICE:
        return _numpy_fallback(feats, pos_table, word_lens, pos, seq_len)
    KW = int(wlen_u.max())

    pos_bf = pos_table_np.astype(BF16)

    in_maps = []
    host_meta = []
    for core in range(N_CORES):
        bs = slice(core * BPC, (core + 1) * BPC)
        feats_cat = np.zeros((BPC * S + PAD_ROWS, D), BF16)
        feats_cat[:BPC * S] = feats[bs].reshape(-1, D).astype(BF16)

        offs = np.zeros((P, BPC * C), np.int32)
        m_all = np.zeros((P, BPC * C * KW), np.float32)
        onehot = np.zeros((PV, BPC * W), BF16)
        for bl in range(BPC):
            bg = core * BPC + bl
            perm = perms[bg]
            st = starts[bg][perm]
            ln = lens[bg][perm]
            rc = recip[bg][perm]
            for c in range(C):
                slot = slice(c * P, (c + 1) * P)
                offs[:, bl * C + c] = bl * S + st[slot]
                wl = int(wlen_u[bl, c])
                k = np.arange(wl)
                col0 = (bl * C + c) * KW
                m_all[:, col0:col0 + wl] = np.where(
                    k[None, :] < ln[slot, None], rc[slot, None], 0.0
                )
            slot_pos = po[bg][perm]
            onehot[slot_pos, bl * W + np.arange(W)] = BF16(1.0)

        in_maps.append({
            "feats_cat": feats_cat,
            "pos_tab": pos_bf,
            "offs": offs,
            "m_all": m_all,
            "onehot": onehot,
        })
        host_meta.append(perms[bs])

    # ---- device program --------------------------------------------------
    from concourse import bass, bacc, mybir
    import concourse.tile as tile

    nc = bacc.Bacc("TRN2", target_bir_lowering=False, debug=False)
    t_feats = nc.dram_tensor("feats_cat", [BPC * S + PAD_ROWS, D],
                             mybir.dt.bfloat16, kind="ExternalInput")
    t_pos = nc.dram_tensor("pos_tab", [PV, D], mybir.dt.bfloat16,
                           kind="ExternalInput")
    t_off = nc.dram_tensor("offs", [P, BPC * C], mybir.dt.int32,
                           kind="ExternalInput")
    t_m = nc.dram_tensor("m_all", [P, BPC * C * KW], mybir.dt.float32,
                         kind="ExternalInput")
    t_oh = nc.dram_tensor("onehot", [PV, BPC * W], mybir.dt.bfloat16,
                          kind="ExternalInput")
    t_out = nc.dram_tensor("out", [BPC * C, P, D], mybir.dt.bfloat16,
                           kind="ExternalOutput")

    with tile.TileContext(nc) as tc:
        with (
            tc.tile_pool(name="const", bufs=1) as cpool,
            tc.tile_pool(name="gath", bufs=4) as gpool,
            tc.tile_pool(name="accp", bufs=3) as apool,
            tc.tile_pool(name="outp", bufs=4) as opool,
            tc.tile_pool(name="psum", bufs=4, space="PSUM") as ppool,
        ):
            pos_sb = cpool.tile([PV, D], mybir.dt.bfloat16)
            oh_sb = cpool.tile([PV, BPC * W], mybir.dt.bfloat16)
            m_sb = cpool.tile([P, BPC * C * KW], mybir.dt.float32)
            off_sb = cpool.tile([P, BPC * C], mybir.dt.int32)
            nc.sync.dma_start(out=off_sb[:], in_=t_off[:])
            nc.sync.dma_start(out=m_sb[:], in_=t_m[:])
            nc.sync.dma_start(out=pos_sb[:], in_=t_pos[:])
            nc.sync.dma_start(out=oh_sb[:], in_=t_oh[:])

            for bl in range(BPC):
                for c in range(C):
                    wl = int(wlen_u[bl, c])
                    col = bl * C + c
                    g = gpool.tile([P, KW * D], mybir.dt.bfloat16, tag="g")
                    nc.gpsimd.indirect_dma_start(
                        out=g[:, 0:wl * D],
                        out_offset=None,
                        in_=t_feats[:],
                        in_offset=bass.IndirectOffsetOnAxis(
                            ap=off_sb[:, col:col + 1], axis=0
                        ),
                    )
                    psum = ppool.tile([P, D], mybir.dt.float32, space="PSUM",
                                      tag="psum")
                    lhs = oh_sb[:, bl * W + c * P: bl * W + (c + 1) * P]
                    nc.tensor.matmul(out=psum[:, 0:512], lhsT=lhs,
                                     rhs=pos_sb[:, 0:512], start=True, stop=True)
                    nc.tensor.matmul(out=psum[:, 512:D], lhsT=lhs,
                                     rhs=pos_sb[:, 512:D], start=True, stop=True)

                    obuf = opool.tile([P, D], mybir.dt.bfloat16, tag="o")
                    mc = col * KW
                    if wl == 1:
                        nc.vector.scalar_tensor_tensor(
                            out=obuf[:], in0=g[:, 0:D],
                            scalar=m_sb[:, mc:mc + 1], in1=psum[:],
                            op0=mybir.AluOpType.mult, op1=mybir.AluOpType.add,
                        )
                    else:
                        acc = apool.tile([P, D], mybir.dt.float32, tag="acc")
                        nc.vector.scalar_tensor_tensor(
                            out=acc[:], in0=g[:, 0:D],
                            scalar=m_sb[:, mc:mc + 1], in1=psum[:],
                            op0=mybir.AluOpType.mult, op1=mybir.AluOpType.add,
                        )
                        for k in range(1, wl - 1):
                            nc.vector.scalar_tensor_tensor(
                                out=acc[:], in0=g[:, k * D:(k + 1) * D],
                                scalar=m_sb[:, mc + k:mc + k + 1], in1=acc[:],
                                op0=mybir.AluOpType.mult, op1=mybir.AluOpType.add,
                            )
                        nc.vector.scalar_tensor_tensor(
                            out=obuf[:], in0=g[:, (wl - 1) * D:wl * D],
                            scalar=m_sb[:, mc + wl - 1:mc + wl], in1=acc[:],
                            op0=mybir.AluOpType.mult, op1=mybir.AluOpType.add,
                        )
                    nc.sync.dma_start(out=t_out[col], in_=obuf[:])
    nc.finalize()

    res = _run_spmd(nc, in_maps, list(range(N_CORES)))
    LAST_RESULTS = res

    out = np.empty((B, W, D), np.float32)
    for core in range(N_CORES):
        arr = np.asarray(res.results[core]["out"])     # [BPC*C, P, D] bf16
        arrf = arr.astype(np.float32)
        for bl in range(BPC):
            slots = arrf[bl * C:(bl + 1) * C].reshape(W, D)
            perm = host_meta[core][bl]
            out[core * BPC + bl][perm] = slots
    return out
